# revision 3
# baseline (speedup 1.0000x reference)
"""EulerAttention Trainium2 kernel.

Full inputs -> full outputs; internally shards batch B=16 across 8 NeuronCores
(2 batches per core). Single Bass program run SPMD via run_bass_kernel_spmd.

Math (per batch b, per head h, s in [0,4096), d in [0,64)):
    theta_q = x/(1+|w_q|) + b_q + t*phi          -> rq = RNE(c1*theta_q)   (host, tiny)
    theta_k = cs/(1+|w_k|) + b_k                 -> rk = RNE(c1*theta_k)   (device)
    score[h,s] = sum_d cos(c2*(rq - rk)) / sqrt(2*dh)
    weights = softmax_s(score);  context[m] = sum_h sum_s weights*cs[s,m]
    out = out_scale * (cos(theta_o^) + sin(theta_o^)),  theta_o^ = LUT-quantized

Device pipeline per 128-row s-tile (natural [s, d] layout):
    t_a = cs * (c1*A)_row                        (DVE tensor_tensor mult)
    ty  = t_a - (rq + 1024 - MAGIC)_row          (DVE TT sub; rounds to int+MAGIC)
    vi  = bits(ty) AND 4095                      (DVE tensor_scalar int)
    cd  = Sin(c2*vi - pi)  == cos(theta_q^-theta_k^)   (ACT)
    score_tile = segmented sum over d (16 heads) (DVE tensor_reduce)
Then per batch: exp (ACT, no max needed - scores bounded by +-5.66), PE-accumulated
context numerator/denominator, normalize, head-sum via PE, exact LUT output stage.
"""
import sys, math, os, tempfile

sys.path.insert(0, "/opt/trn_rl_repo")
import numpy as np

B, S, D, H = 16, 4096, 1024, 16
DH = D // H                      # 64
N_CORES = 8
B_LOC = B // N_CORES             # 2
N_ST = S // 128                  # 32 s-tiles per batch
LUT = 4096
PHI = (1.0 + math.sqrt(5.0)) / 2.0
TWO_PI = 2.0 * math.pi
C1 = np.float32(LUT / TWO_PI)
C2 = np.float32(TWO_PI / LUT)
MAGIC = np.float32(1.5 * 2 ** 23)        # 12582912, divisible by 4096
PI_F = float(np.float32(math.pi))
INV_TAU = float(np.float32(1.0 / math.sqrt(2.0 * DH)))

_CACHE = {}


def _reg_const(nc, mybir, val):
    val = float(val)
    t = nc.alloc_sbuf_tensor(f"const-f32-{val}", [128, 1], mybir.dt.float32)
    nc.gpsimd.memset(t.ap(), val)
    nc.const_aps.aps[(mybir.dt.float32, val)] = t.ap()


def build_program():
    from concourse import bacc, tile, mybir

    nc = bacc.Bacc("TRN2", target_bir_lowering=False, debug=False)
    _reg_const(nc, mybir, -PI_F)
    nc.all_engine_barrier()

    f32 = mybir.dt.float32
    i32 = mybir.dt.int32
    AT = mybir.ActivationFunctionType
    OP = mybir.AluOpType

    cs_d = nc.dram_tensor("cs", [B_LOC, S, D], f32, kind="ExternalInput").ap()
    c1a_d = nc.dram_tensor("c1a", [128, D], f32, kind="ExternalInput").ap()
    rq_d = nc.dram_tensor("rqrow", [B_LOC, 128, D], f32, kind="ExternalInput").ap()
    ones_d = nc.dram_tensor("ones", [128, 1], f32, kind="ExternalInput").ap()
    rot_d = nc.dram_tensor("roT", [128, 8], f32, kind="ExternalInput").ap()
    bot_d = nc.dram_tensor("boT", [B_LOC, 128, 8], f32, kind="ExternalInput").ap()
    ost_d = nc.dram_tensor("osT", [128, 8], f32, kind="ExternalInput").ap()
    out_d = nc.dram_tensor("out", [B_LOC, D], f32, kind="ExternalOutput").ap()

    with tile.TileContext(nc) as tc:
        from contextlib import ExitStack
        with ExitStack() as ctx:
            cpool = ctx.enter_context(tc.tile_pool(name="cs", bufs=N_ST))
            wpool = ctx.enter_context(tc.tile_pool(name="work", bufs=2))
            spool = ctx.enter_context(tc.tile_pool(name="small", bufs=2))
            kpool = ctx.enter_context(tc.tile_pool(name="konst", bufs=1))
            ppool = ctx.enter_context(tc.tile_pool(name="psum", bufs=1, space="PSUM"))

            c1a = kpool.tile([128, D], f32, name="c1a_t")
            nc.sync.dma_start(c1a[:, :], c1a_d)
            rqr = [kpool.tile([128, D], f32, name=f"rqr{b}") for b in range(B_LOC)]
            for b in range(B_LOC):
                nc.sync.dma_start(rqr[b][:, :], rq_d[b])
            ones = kpool.tile([128, 1], f32, name="ones_t")
            nc.sync.dma_start(ones[:, :], ones_d)
            rot = kpool.tile([128, 8], f32, name="rot_t")
            nc.sync.dma_start(rot[:, :], rot_d)
            bot = [kpool.tile([128, 8], f32, name=f"bot{b}") for b in range(B_LOC)]
            for b in range(B_LOC):
                nc.sync.dma_start(bot[b][:, :], bot_d[b])
            ost = kpool.tile([128, 8], f32, name="ost_t")
            nc.sync.dma_start(ost[:, :], ost_d)

            for b in range(B_LOC):
                cs_tiles = []
                scores = spool.tile([128, N_ST * H], f32, name="scores", tag="scores")
                for st in range(N_ST):
                    cst = cpool.tile([128, D], f32, name=f"cs_{b}_{st}", tag="cs")
                    nc.sync.dma_start(cst[:, :], cs_d[b, st * 128:(st + 1) * 128, :])
                    cs_tiles.append(cst)
                    ta = wpool.tile([128, D], f32, name=f"ta_{b}_{st}", tag="ta")
                    nc.vector.tensor_tensor(ta[:, :], cst[:, :], c1a[:, :], OP.mult)
                    ty = wpool.tile([128, D], f32, name=f"ty_{b}_{st}", tag="ty")
                    nc.vector.tensor_tensor(ty[:, :], ta[:, :], rqr[b][:, :], OP.subtract)
                    vi = wpool.tile([128, D], i32, name=f"vi_{b}_{st}", tag="vi")
                    nc.vector.tensor_scalar(vi[:, :], ty[:, :].bitcast(i32),
                                            4095, None, OP.bitwise_and)
                    cd = wpool.tile([128, D], f32, name=f"cd_{b}_{st}", tag="cd")
                    nc.scalar.activation(cd[:, :], vi[:, :], AT.Sin,
                                         bias=-PI_F, scale=float(C2))
                    nc.vector.tensor_reduce(
                        scores[:, st * H:(st + 1) * H],
                        cd[:, :].rearrange("p (h d) -> p h d", h=H),
                        mybir.AxisListType.X, OP.add)

                # phase 2: softmax-free streaming attention (scores bounded)
                p = spool.tile([128, N_ST * H], f32, name="p", tag="p")
                nc.scalar.activation(p[:, :], scores[:, :], AT.Exp, scale=INV_TAU)

                num0 = ppool.tile([16, 512], f32, name=f"num0_{b}", tag="num0")
                num1 = ppool.tile([16, 512], f32, name=f"num1_{b}", tag="num1")
                den = ppool.tile([16, 1], f32, name=f"den_{b}", tag="den")
                for st in range(N_ST):
                    lhs = p[:, st * H:(st + 1) * H]
                    kw = dict(start=(st == 0), stop=(st == N_ST - 1))
                    nc.tensor.matmul(num0[:, :], lhs, cs_tiles[st][:, 0:512], **kw)
                    nc.tensor.matmul(num1[:, :], lhs, cs_tiles[st][:, 512:1024], **kw)
                    nc.tensor.matmul(den[:, :], lhs, ones[:, :], **kw)

                rec = spool.tile([16, 1], f32, name=f"rec_{b}", tag="rec")
                nc.vector.reciprocal(rec[:, :], den[:, :])
                ctxn = spool.tile([16, D], f32, name=f"ctxn_{b}", tag="ctxn")
                nc.vector.tensor_scalar(ctxn[:, 0:512], num0[:, :], rec[:, :], None, OP.mult)
                nc.vector.tensor_scalar(ctxn[:, 512:1024], num1[:, :], rec[:, :], None, OP.mult)

                # head-sum -> ctxT [128, 8] with m = c*128 + p
                ctxT = ppool.tile([128, 8], f32, name=f"ctxT_{b}", tag="ctxT")
                for c in range(8):
                    nc.tensor.matmul(ctxT[:, c:c + 1],
                                     ctxn[:, c * 128:(c + 1) * 128],
                                     ones[0:16, :], start=True, stop=True)

                # output stage (exact LUT quantization), all [128, 8]
                m1 = spool.tile([128, 8], f32, name=f"m1_{b}", tag="m1")
                nc.vector.tensor_tensor(m1[:, :], ctxT[:, :], rot[:, :], OP.mult)
                m2 = spool.tile([128, 8], f32, name=f"m2_{b}", tag="m2")
                nc.vector.tensor_tensor(m2[:, :], m1[:, :], bot[b][:, :], OP.add)
                yo = spool.tile([128, 8], f32, name=f"yo_{b}", tag="yo")
                nc.vector.tensor_scalar(yo[:, :], m2[:, :],
                                        float(MAGIC) + 2560.0, None, OP.add)
                vo = spool.tile([128, 8], i32, name=f"vo_{b}", tag="vo")
                nc.vector.tensor_scalar(vo[:, :], yo[:, :].bitcast(i32),
                                        4095, None, OP.bitwise_and)
                sp = spool.tile([128, 8], f32, name=f"sp_{b}", tag="sp")
                nc.scalar.activation(sp[:, :], vo[:, :], AT.Sin,
                                     bias=-PI_F, scale=float(C2))
                ot = spool.tile([128, 8], f32, name=f"ot_{b}", tag="ot")
                nc.vector.tensor_tensor(ot[:, :], sp[:, :], ost[:, :], OP.mult)
                nc.sync.dma_start(out_d[b].rearrange("(c p) -> p c", p=128), ot[:, :])

    nc.compile()
    return nc


def _host_prep(x, t, w_query, b_query, w_key, b_key, w_out, b_out, out_scale):
    f = np.float32
    # query indices rq [B, D] (exact f32 replication of reference order)
    xh = x.reshape(B, H, DH).astype(f)
    t_phi = (t.astype(f) * f(PHI)).astype(f)
    theta_q = ((xh / (f(1.0) + np.abs(w_query.astype(f)))).astype(f)
               + b_query.astype(f)).astype(f)
    theta_q = (theta_q + t_phi[:, None, None]).astype(f)
    rq = np.round((theta_q * C1).astype(f)).astype(np.float64).reshape(B, D)

    a_key = 1.0 / (1.0 + np.abs(w_key.astype(np.float64)))
    c1a_row = (float(C1) * a_key).reshape(D).astype(f)        # [D]
    c1a = np.broadcast_to(c1a_row, (128, D)).copy()
    # b_key support: fold c1*b_key into the row subtracted before rounding
    c1bk = (float(C1) * b_key.astype(np.float64)).reshape(D)
    rq_row = (rq + 1024.0 - np.float64(MAGIC) - c1bk[None, :])  # [B, D]
    rq_rep = np.broadcast_to(rq_row[:, None, :].astype(f), (B, 128, D)).copy()

    ones = np.ones((128, 1), f)
    # output-stage constants, m = c*128 + p
    m_idx = (np.arange(8)[None, :] * 128 + np.arange(128)[:, None])  # [128, 8]
    a_out = (1.0 / (1.0 + np.abs(w_out.astype(np.float64)))).reshape(D)
    roT = (float(C1) * a_out[m_idx]).astype(f)
    t_phi64 = t_phi.astype(np.float64)
    boT = np.empty((B, 128, 8), f)
    for b in range(B):
        boT[b] = (float(C1) * (b_out.astype(np.float64).reshape(D)[m_idx]
                               + t_phi64[b])).astype(f)
    osT = (out_scale.astype(np.float64).reshape(D)[m_idx]
           * math.sqrt(2.0)).astype(f)
    return c1a, rq_rep, ones, roT, boT, osT


def kernel(x, cached_states, t, w_query, b_query, w_key, b_key, w_out, b_out,
           out_scale):
    from concourse.bass_utils import run_bass_kernel_spmd

    x = np.asarray(x); cached_states = np.asarray(cached_states); t = np.asarray(t)
    c1a, rq_rep, ones, roT, boT, osT = _host_prep(
        x, t, w_query, b_query, w_key, b_key, w_out, b_out, out_scale)

    if "nc" not in _CACHE:
        _CACHE["nc"] = build_program()
    nc = _CACHE["nc"]

    in_maps = []
    for c in range(N_CORES):
        b0 = c * B_LOC
        in_maps.append({
            "cs": np.ascontiguousarray(cached_states[b0:b0 + B_LOC]).astype(np.float32),
            "c1a": c1a,
            "rqrow": np.ascontiguousarray(rq_rep[b0:b0 + B_LOC]),
            "ones": ones,
            "roT": roT,
            "boT": np.ascontiguousarray(boT[b0:b0 + B_LOC]),
            "osT": osT,
        })
    res = run_bass_kernel_spmd(nc, in_maps, core_ids=list(range(N_CORES)))
    _CACHE["last_results"] = res
    out = np.concatenate([res.results[c]["out"] for c in range(N_CORES)], axis=0)
    return out.astype(np.float32)


if __name__ == "__main__":
    # quick self-run with random data (no reference comparison)
    rng = np.random.default_rng(0)
    ins = {
        "x": rng.standard_normal((B, D), dtype=np.float32),
        "cached_states": rng.standard_normal((B, S, D), dtype=np.float32),
        "t": rng.random(B, dtype=np.float32),
        "w_query": (rng.standard_normal((H, DH)) * 0.02).astype(np.float32),
        "b_query": np.zeros((H, DH), np.float32),
        "w_key": (rng.standard_normal((H, DH)) * 0.02).astype(np.float32),
        "b_key": np.zeros((H, DH), np.float32),
        "w_out": (rng.standard_normal(D) * 0.02).astype(np.float32),
        "b_out": np.zeros(D, np.float32),
        "out_scale": (np.ones(D) * 0.5).astype(np.float32),
    }
    o = kernel(**ins)
    print("out", o.shape, o.dtype, float(np.abs(o).mean()))
